# revision 33
# baseline (speedup 1.0000x reference)
"""GAT message-passing kernel for Trainium2 (8 NeuronCores, SPMD).

Problem (per full input):
    B=8, S=512, N=32 neighbors, H=256, V=100001
    out[b,s,:] = sum_n softmax_n(leakyrelu(a_w . [src, cand_n]) + mask*NEG) * cand_n
    candidates = [self] + 32 neighbors (self never masked)

Sharding: data-parallel over B — core c handles batch row c with a
per-core deduplicated slice of the embedding table.

Per-core algorithm (s-tiles of 128 nodes, 4 tiles; 133us baseline -> ~87us):
    - host compacts each node's unmasked neighbors into the leading slots;
      pad slots index an appended table row r = c*awc with c = NEG/|awc|^2,
      so a pad's logit is exactly NEG and its softmax weight underflows to
      0.0 — no mask tensor on device at all
    - host dedups each core's candidate ids into a local table T_c
      (~8.6K rows << 32767 so int16-addressable), remaps cands to local
      ids; the device gathers 128*GS rows per dma_gather instruction.
      SWDGE descgen costs ~8ns/descriptor ON THE POOL ENGINE when issued
      on one queue (the old per-slot indirect DMA path burned 86us there);
      rotating gathers across 4 SWDGE queues lets up to 4 descgens run
      concurrently on the Q7 cluster, and SCRATCH > 16384 lets a queue
      start the next batch's descgen while the previous one drains
    - the table is cast to bf16 on host: halves gather traffic, doubles
      DVE/PE throughput; fp32 accumulation keeps rel err ~2.4e-3
    - logits z[:,n] = sum_h F[p,n,h]*awc[h] via per-slot STT accum_out on
      Vector (the only engine with reduce-capable elementwise ops);
      zl = prelu(z+zsrc) and e = exp(zl) (+group denominators) on Scalar —
      Prelu, unlike Lrelu, shares the exp_and_others act table with Exp so
      no 1.3us table reload per group; no max-subtraction (logits tiny;
      pads underflow to exactly 0)
    - aggregation sum_n diag(e_n) @ F_n accumulates in PSUM via bf16
      matmuls; diag builds run on Scalar inline, on Vector delayed one
      group (dodges head-of-line stalls of later logit STTs on the
      in-order DVE), and on Pool only for late tiles (after the gather
      stream leaves the Pool queue); 1/sum(e) folds into the
      PSUM-evacuation scale on ScalarE; a_w/a_b land as single-descriptor
      DMAs replicated on-chip via partition_broadcast (a 128-partition
      broadcast DMA would cost ~3.5us of descriptor time up front)
"""

import numpy as np

B, S, N, H, V = 8, 512, 32, 256, 100001
NC1 = N + 1  # 33 candidate slots (self + neighbors)
P = 128
S_TILES = S // P
NEG = -1.0e9
SLOPE = 0.2
N_CORES = 8

# Tuning knobs
GS = 8            # gather group size (slots per dma_gather; 128*GS <= 1024
                  # descriptors — the dynamic-DMA ring carveout; smaller
                  # groups (odd sizes / 2-in-flight per ring) misbehave on HW
DG_PATTERN = ("sv", "sv", "sp", "pp")  # per-tile diag-build engine cycle:
                      # s=Scalar (inline after exp), v=Vector (delayed one
                      # group to dodge head-of-line stalls on the in-order
                      # DVE), p=Pool — only usable for late tiles, after the
                      # gather descgen stream has left the Pool queue
NQ = 4            # SWDGE queues; rotate gathers across them
SCRATCH = 49152   # dynamic-DMA descriptor scratch; 3072 descs -> multiple
                  # 1024-desc gathers in flight per ring, letting descgen
                  # overlap the previous batch's drain
FIRST_SMALL = (2, 4)  # leading group sizes of tile 0: feed Vector early
EMB_BF16 = True   # gather/aggregate in bf16 (half DMA traffic)
USE_LRELU = True  # Scalar Lrelu (not implemented in CoreSim; False = V max-trick)

_CACHE: dict = {}


def _groups(ncc, t=1):
    lead = [s for s in (FIRST_SMALL if t == 0 else ())]
    base = 0
    gs = []
    for s in lead:
        if base + s >= ncc:
            break
        gs.append((base, base + s))
        base += s
    rest = ncc - base
    k = max(1, -(-rest // GS))
    bs = [base + round(i * rest / k) for i in range(k + 1)]
    gs += [(bs[i], bs[i + 1]) for i in range(k) if bs[i + 1] > bs[i]]
    return gs


def _build_nc(ncc_list, n_uniq, emb_bf16):
    import concourse.bacc as bacc
    import concourse.mybir as mybir
    import concourse.tile as tile
    from concourse.masks import make_identity

    f32 = mybir.dt.float32
    i16 = mybir.dt.int16
    dt_e = mybir.dt.bfloat16 if emb_bf16 else f32
    Alu = mybir.AluOpType
    Act = mybir.ActivationFunctionType
    X = mybir.AxisListType.X

    nc = bacc.Bacc(
        "TRN2",
        target_bir_lowering=False,
        debug=False,
        enable_asserts=False,
        num_devices=N_CORES,
        num_swdge_queues=NQ,
        dynamic_dma_scratch_size=SCRATCH,
    )
    nc._gq = 0

    ncc_sum = sum(ncc_list)
    emb_d = nc.dram_tensor("emb_table", [n_uniq, H], dt_e, kind="ExternalInput").ap()
    gidx_d = nc.dram_tensor("gidx", [P, 8 * ncc_sum], i16, kind="ExternalInput").ap()
    aw_d = nc.dram_tensor("a_w", [2, H], f32, kind="ExternalInput").ap()
    ab_d = nc.dram_tensor("a_b", [1, 1], f32, kind="ExternalInput").ap()
    out_d = nc.dram_tensor("out", [S, H], f32, kind="ExternalOutput").ap()

    with tile.TileContext(nc) as tc:
        with (
            tc.tile_pool(name="cpool", bufs=1) as cpool,
            tc.tile_pool(name="fpool", bufs=1) as fpool,
            tc.tile_pool(name="spool", bufs=2) as spool,
            tc.tile_pool(name="dpool", bufs=16) as dpool,
            tc.tile_pool(name="ppool", bufs=4, space="PSUM") as ppool,
        ):
            # ---- constants (once) ----
            ident = cpool.tile([P, P], dt_e)
            make_identity(nc, ident)

            # gidx first: it gates the first gather
            gidx = cpool.tile([P, 8 * ncc_sum], i16)
            nc.sync.dma_start(out=gidx[:], in_=gidx_d)

            # a_w/a_b land as single-descriptor DMAs on partition 0 and are
            # replicated on-chip (a 128-wide broadcast DMA costs ~3.5us of
            # descriptor traffic that would delay the gather stream)
            aw_lin = cpool.tile([1, 2 * H], f32)
            nc.sync.dma_start(
                out=aw_lin[:], in_=aw_d.rearrange("a h -> (a h)").unsqueeze(0)
            )
            ab_lin = cpool.tile([1, 1], f32)
            nc.sync.dma_start(out=ab_lin[:], in_=ab_d)
            aw_rep = cpool.tile([P, 2 * H], f32)
            nc.gpsimd.partition_broadcast(aw_rep[:], aw_lin[:])
            ab_rep = cpool.tile([P, 1], f32)
            nc.gpsimd.partition_broadcast(ab_rep[:], ab_lin[:])
            if emb_bf16:
                aw_rep_e = cpool.tile([P, 2 * H], dt_e)
                nc.gpsimd.tensor_copy(aw_rep_e[:], aw_rep[:])
            else:
                aw_rep_e = aw_rep
            aws_rep = aw_rep_e[:, 0:H]
            awc_rep = aw_rep_e[:, H : 2 * H]

            off = [0]
            for t in range(S_TILES):
                off.append(off[-1] + ncc_list[t])

            F_all = fpool.tile([P, ncc_sum * H], dt_e)

            def F3_of(t):
                return F_all[:, off[t] * H : off[t + 1] * H].rearrange(
                    "p (n h) -> p n h", n=ncc_list[t]
                )

            def issue_gathers(t):
                F3 = F3_of(t)
                for a, b in _groups(ncc_list[t], t):
                    g = b - a
                    nc.gpsimd.dma_gather(
                        out_ap=F3[:, a:b, :],
                        in_ap=emb_d,
                        idxs_ap=gidx[:, 8 * (off[t] + a) : 8 * (off[t] + b)],
                        num_idxs=P * g,
                        num_idxs_reg=P * g,
                        elem_size=H,
                        queue_num=nc._gq % NQ,
                    )
                    nc._gq += 1

            # all gathers up front: the 4 SWDGE rings stream back-to-back
            # and all other Pool work is kept off the queue behind them
            dg_cycle = 0
            for t in range(S_TILES):
                issue_gathers(t)
            for t in range(S_TILES):
                ncc = ncc_list[t]
                GROUPS = _groups(ncc, t)
                rows = slice(t * P, (t + 1) * P)
                F3 = F3_of(t)

                trash_v = spool.tile([P, H], dt_e)
                zsrc = spool.tile([P, 1], f32)
                z = spool.tile([P, ncc], f32)
                zl = spool.tile([P, ncc], f32)
                e = spool.tile([P, ncc], f32)
                deng = spool.tile([P, len(GROUPS)], f32)
                acc = ppool.tile([P, H], f32)

                mm_cnt = 0
                pend_v: list = []

                def emit_mm(n, dg, _acc=acc, _F3=F3, _ncc=ncc):
                    nonlocal mm_cnt
                    nc.tensor.matmul(
                        out=_acc[:],
                        lhsT=dg[:],
                        rhs=_F3[:, n, :],
                        start=(mm_cnt == 0),
                        stop=(mm_cnt == _ncc - 1),
                    )
                    mm_cnt += 1

                def flush_v(_e=e):
                    # V-side diag builds for the PREVIOUS group: by now the
                    # Scalar exp for it has long finished, so the in-order
                    # DVE doesn't stall its later logit STTs behind them
                    for n in pend_v:
                        dg = dpool.tile([P, P], dt_e, name="dg")
                        nc.vector.tensor_scalar_mul(
                            dg[:], ident[:], _e[:, n : n + 1]
                        )
                        emit_mm(n, dg)
                    pend_v.clear()

                for gi, (a, b) in enumerate(GROUPS):
                    for n in range(a, b):
                        nc.vector.scalar_tensor_tensor(
                            out=trash_v[:],
                            in0=F3[:, n, :],
                            scalar=1.0,
                            in1=awc_rep,
                            op0=Alu.mult,
                            op1=Alu.mult,
                            accum_out=z[:, n : n + 1],
                        )
                    flush_v()
                    if gi == 0:
                        zsrc_raw = spool.tile([P, 1], f32)
                        nc.vector.scalar_tensor_tensor(
                            out=trash_v[:],
                            in0=F3[:, 0, :],
                            scalar=1.0,
                            in1=aws_rep,
                            op0=Alu.mult,
                            op1=Alu.mult,
                            accum_out=zsrc_raw[:],
                        )
                        nc.vector.tensor_scalar_add(zsrc[:], zsrc_raw[:], ab_rep[:])

                    zg = zl[:, a:b]
                    if USE_LRELU:
                        # zl = lrelu(z + zsrc) in one Scalar op; Prelu
                        # (parametric_relu) lives in the exp_and_others
                        # act table so no table reload vs Exp (Lrelu does
                        # not and costs a 1.3us ACT_TABLE_LOAD per switch)
                        nc.scalar.activation(
                            zg,
                            z[:, a:b],
                            Act.Prelu,
                            bias=zsrc[:],
                            scale=1.0,
                            alpha=SLOPE,
                        )
                    else:
                        nc.vector.tensor_scalar_add(zg, z[:, a:b], zsrc[:])
                        nc.vector.tensor_scalar_mul(z[:, a:b], zg, SLOPE)
                        nc.vector.tensor_max(zg, zg, z[:, a:b])
                    nc.scalar.activation(
                        e[:, a:b],
                        zg,
                        Act.Exp,
                        accum_out=deng[:, gi : gi + 1],
                    )
                    pat = DG_PATTERN[t]
                    for n in range(a, b):
                        ch = pat[dg_cycle % len(pat)]
                        dg_cycle += 1
                        if ch == "p":
                            # diag(e_n) in one Pool op: e on the diagonal,
                            # 0 elsewhere
                            dg = dpool.tile([P, P], dt_e, name="dg")
                            nc.gpsimd.affine_select(
                                out=dg[:],
                                in_=e[:, n : n + 1].to_broadcast([P, P]),
                                compare_op=Alu.is_equal,
                                fill=0.0,
                                base=0,
                                pattern=[[-1, P]],
                                channel_multiplier=1,
                            )
                            emit_mm(n, dg)
                        elif ch == "s":
                            dg = dpool.tile([P, P], dt_e, name="dg")
                            nc.scalar.mul(dg[:], ident[:], e[:, n : n + 1])
                            emit_mm(n, dg)
                        else:
                            pend_v.append(n)
                flush_v()

                den = spool.tile([P, 1], f32)
                nc.vector.tensor_reduce(den[:], deng[:], axis=X, op=Alu.add)
                rden = spool.tile([P, 1], f32)
                nc.vector.reciprocal(rden[:], den[:])
                o = spool.tile([P, H], f32)
                nc.scalar.mul(o[:], acc[:], rden[:])
                nc.sync.dma_start(out=out_d[rows, :], in_=o[:])

    nc.compile()
    return nc


def _get_nc(ncc_list, n_uniq):
    key = (tuple(ncc_list), n_uniq, EMB_BF16, USE_LRELU, GS, DG_PATTERN, NQ, SCRATCH, FIRST_SMALL)
    if key not in _CACHE:
        _CACHE[key] = _build_nc(tuple(ncc_list), n_uniq, EMB_BF16)
    return _CACHE[key]


def _ensure_axon_hooks():
    """Provide antenv.axon_hooks if the image lacks it, so trace=True /
    BASS_TRACE=1 profiling requests don't crash run_bass_kernel_spmd."""
    import sys
    import types

    try:
        import antenv.axon_hooks  # noqa: F401

        return
    except ImportError:
        pass
    try:
        import antenv
    except ImportError:
        return
    mod = types.ModuleType("antenv.axon_hooks")
    state = {"hook": None}

    def set_axon_ntff_profile_hook(h):
        state["hook"] = h

    def get_axon_ntff_profile_hook():
        if state["hook"] is None:
            try:
                from trn_agent_boot.trn_boot import _ntff_profile_via_ctypes

                state["hook"] = _ntff_profile_via_ctypes("/opt/axon/libaxon_pjrt.so")
            except Exception:
                return None
        return state["hook"]

    mod.set_axon_ntff_profile_hook = set_axon_ntff_profile_hook
    mod.get_axon_ntff_profile_hook = get_axon_ntff_profile_hook
    sys.modules["antenv.axon_hooks"] = mod
    antenv.axon_hooks = mod


def _prep_host(inputs):
    """Compact unmasked neighbors to the leading slots (pads index an
    appended row that forces logit == NEG), dedup each core's candidate
    ids into a local int16-addressable table, and build the wrapped
    dma_gather index arrays."""
    node_ids = np.asarray(inputs["node_ids"]).astype(np.int32).reshape(B, S)
    neighs = np.asarray(inputs["neighs"]).astype(np.int32).reshape(B, S, N)
    mask = np.asarray(inputs["mask"]).astype(np.int32).reshape(B, S, N)
    emb = np.ascontiguousarray(np.asarray(inputs["emb_table"], dtype=np.float32))
    a_w = np.ascontiguousarray(np.asarray(inputs["a_w"], dtype=np.float32).reshape(2, H))
    a_b = np.ascontiguousarray(np.asarray(inputs["a_b"], dtype=np.float32).reshape(1, 1))

    un_cnt = (mask == 0).sum(axis=-1)  # [B, S]
    # sort nodes by unmasked count (desc) so later tiles need fewer slots
    perm = np.argsort(-un_cnt, axis=1, kind="stable")  # [B, S]
    nid_p = np.take_along_axis(node_ids, perm, axis=1)
    nbr_p = np.take_along_axis(neighs, perm[..., None], axis=1)
    msk_p = np.take_along_axis(mask, perm[..., None], axis=1)
    cnt_p = np.take_along_axis(un_cnt, perm, axis=1)

    cnt_t = cnt_p.reshape(B, S_TILES, P)
    ncc_list = [max(int(cnt_t[:, t, :].max()) + 1, 2) for t in range(S_TILES)]
    ncc = max(ncc_list)
    order = np.argsort(msk_p, axis=-1, kind="stable")  # unmasked first
    sneighs = np.take_along_axis(nbr_p, order, axis=-1)
    cands = np.empty((B, S, ncc), np.int32)
    cands[..., 0] = nid_p
    cands[..., 1:] = sneighs[..., : ncc - 1]
    ks = np.arange(1, ncc)[None, None, :]
    cands[..., 1:][ks > cnt_p[..., None]] = V  # pad slots -> appended row

    # appended pad row r with dot(r, awc) == NEG exactly
    awc = a_w[1]
    pad_row = (NEG / max(float(awc @ awc), 1e-30)) * awc
    emb_aug = np.concatenate([emb, pad_row[None, :].astype(np.float32)], axis=0)

    # per-core dedup: local table + int16 local ids
    uniqs, lcands = [], []
    for c in range(N_CORES):
        u = np.unique(cands[c])
        uniqs.append(u)
        lcands.append(np.searchsorted(u, cands[c]).astype(np.int16))
    n_uniq = max(len(u) for u in uniqs)
    tables = np.zeros((N_CORES, n_uniq, H), np.float32)
    for c in range(N_CORES):
        tables[c, : len(uniqs[c])] = emb_aug[uniqs[c]]

    # wrapped dma_gather index arrays: one [16, 8g] block per slot group,
    # list position i = slot*128 + node so row i lands at F3[i%128, i//128];
    # wrapped as arr[p, s] = list[s*16+p], replicated to all 8 Q7 lanes
    ncc_sum = sum(ncc_list)
    gidx = np.zeros((N_CORES, P, 8 * ncc_sum), np.int16)
    offt = np.cumsum([0] + ncc_list)
    for c in range(N_CORES):
        lc_t = lcands[c].reshape(S_TILES, P, ncc)
        for t in range(S_TILES):
            for a, b in _groups(ncc_list[t], t):
                lst = lc_t[t][:, a:b].T.ravel()  # [g*128], slot-major
                blk = lst.reshape(-1, 16).T  # [16, 8g]
                gidx[c, :, 8 * (offt[t] + a) : 8 * (offt[t] + b)] = np.tile(
                    blk, (8, 1)
                )

    return gidx, tables, n_uniq, a_w, a_b, perm, ncc_list


def kernel(**inputs) -> np.ndarray:
    _ensure_axon_hooks()
    from concourse.bass_utils import run_bass_kernel_spmd

    gidx, tables, n_uniq, a_w, a_b, perm, ncc_list = _prep_host(inputs)
    if EMB_BF16:
        import ml_dtypes

        tables = np.ascontiguousarray(tables.astype(ml_dtypes.bfloat16))

    nc = _get_nc(ncc_list, n_uniq)
    in_maps = [
        {
            "gidx": gidx[c],
            "emb_table": tables[c],
            "a_w": a_w,
            "a_b": a_b,
        }
        for c in range(N_CORES)
    ]
    core_ids = list(range(N_CORES))
    try:
        res = run_bass_kernel_spmd(nc, in_maps, core_ids=core_ids)
    except Exception:
        # transient device wedge — retry once
        res = run_bass_kernel_spmd(nc, in_maps, core_ids=core_ids)
    _CACHE["last_res"] = res
    out = np.empty((N_CORES, S, H), np.float32)
    for c in range(N_CORES):
        out[c, perm[c], :] = res.results[c]["out"]
    return out
